# revision 14
# baseline (speedup 1.0000x reference)
"""Trainium2 Bass kernel for the per-task embedding MLP (embedding_lookup).

Computation (per sample j with task t = task_ids[j]):
    h      = x[j] @ l1_emb[t].reshape(256, 128) + l1_bias[t]
    g      = gelu_exact(h)
    out[j] = sum(g * l2_emb[t]) + l2_bias[t, 0]

Strategy: shard the *task* axis across the 8 cores (125 contiguous tasks per
core), so each core streams its slab of l1_emb exactly once (the memory
roofline).  Samples are routed (host-side index math only) to the core
owning their task and packed into a slot grid of W=8 columns per group
(tasks with more than W samples get extra groups with duplicated weight
rows; tasks with no samples get none), so all 8 cores run one identical
SPMD program: per group, two K=128 matmuls of the task's [256,128] fp8
weights against its [256,W] fp16 x-columns accumulate hT[128, cols] in
PSUM; a per-block "bias matmul" (block's l1_bias rows against a one-hot
group->slot indicator) accumulates b1 into the same PSUM, so the epilogue
is just gelu (ACT from PSUM, fp16 out) -> *w2 (DVE, 16-bit) into a
resident prodt slab; the hidden-dim reduce (ones-vector matmul), +b2 and
the single output DMA all happen once at the end (keeping them out of the
per-block stream avoids PE-FIFO bubbles and semaphore churn).

Precision: w1 is host-cast to fp8 E3M4 (float8e3) — halves the dominant
l1_emb DMA stream vs fp16; x stays fp16 (mixed-dtype matmuls are fine).
b1/w2 ride in fp16, b2 in fp32, PSUM accumulation in fp32.  Measured
end-to-end L2 relative error ~1.3e-2 (dominated by the 4-bit-mantissa
weight quantization), within the 2e-2 gate.

DMA plan: a dma_start costs ~0.6us of issuing-engine time and small-line
descriptors are starved by the per-engine packet round-robin, so
descriptors are few and large.  x is packed block-major (both K-halves of
a block contiguous -> one descriptor per block, 4*cols-byte lines).  The
sync HWDGE ring carries the tiny x lead (blocks 0-1) then the w1 slabs;
the scalar ring carries the packed constants and the x slabs for blocks
2+ in parallel.  All block tiles are SBUF/PSUM-resident (bufs=NB), so no
descriptor ever waits on compute progress.
"""

import numpy as np
import ml_dtypes

import concourse.bacc as bacc
import concourse.mybir as mybir
import concourse.tile as tile
from concourse.bass_utils import run_bass_kernel_spmd

NUM_TASKS = 1000
N_FEATURES = 256
HIDDEN = 128
BATCH = 4096
N_CORES = 8
TPC = NUM_TASKS // N_CORES  # tasks per core = 125

# Module-level knobs for the test harness (the grader just calls kernel()).
W1_DTYPE = "float8e3"  # w1 slab dtype: "float8e3" | "float16"
TRACE = False
TMPDIR = None  # optional fixed artifact dir for profiling runs
SIM_CORES = None  # e.g. [0]: run CoreSim for those cores instead of hardware
SIM_EXECUTOR_CLS = None  # optional InstructionExecutor subclass for CoreSim
LAST_RESULTS = None

_PROGRAM_CACHE = {}

W = 8           # sample slots per group
GB = 60         # max groups per PSUM block (GB*W*4B <= 2KB bank)
MAXG = 20       # max groups per w1 DMA descriptor (5KB lines)


def _np_dtype(name):
    return {
        "float8e3": ml_dtypes.float8_e3m4,
        "float16": np.float16,
        "float32": np.float32,
    }[name]


def _block_sizes(NG):
    # the kernel is DMA-bound: big blocks first (fewer descriptors, fewer
    # ops), a short ramp-down at the end so the post-last-matmul epilogue
    # chain is cheap
    rem = NG - 20
    sizes = [GB] * (rem // GB) + ([rem % GB] if rem % GB else []) + [15, 5]
    assert sum(sizes) == NG and all(s <= GB for s in sizes)
    return sizes


def _build_program(NG, w1_dtype):
    sizes = _block_sizes(NG)
    NB = len(sizes)
    NSLOT = NG * W
    f32 = mybir.dt.float32
    f16 = mybir.dt.float16
    wdt = getattr(mybir.dt, w1_dtype)

    IND = GB * W                      # indicator columns in cstA
    CCA = IND + NB * HIDDEN           # cstA fp16 columns

    def b1off(b):
        return IND + b * HIDDEN

    nc = bacc.Bacc("TRN2", target_bir_lowering=False, debug=False)

    # x is packed block-major: for block b both K-halves sit contiguously
    # ([.., half0 cols, half1 cols, ..]), so one descriptor per block with
    # 4*cols-byte lines
    xT_d = nc.dram_tensor("xTb", [128, 2 * NSLOT], f16, kind="ExternalInput").ap()
    w1_d = nc.dram_tensor(
        "w1s", [NG * N_FEATURES * HIDDEN], wdt, kind="ExternalInput"
    ).ap()
    cstA_d = nc.dram_tensor("cstA", [GB, CCA], f16, kind="ExternalInput").ap()
    cstW_d = nc.dram_tensor("cstW", [128, NG], f16, kind="ExternalInput").ap()
    b2_d = nc.dram_tensor("b2r", [1, NG], f32, kind="ExternalInput").ap()
    out_d = nc.dram_tensor("out", [1, NSLOT], f32, kind="ExternalOutput").ap()

    gelu = mybir.ActivationFunctionType.Gelu

    with tile.TileContext(nc) as tc:
        with (
            tc.tile_pool(name="const", bufs=1) as constp,
            tc.tile_pool(name="w1pool", bufs=1) as w1p,
            tc.tile_pool(name="work", bufs=3) as workp,
            tc.tile_pool(name="hpsum", bufs=NB, space="PSUM") as hpsp,
            tc.tile_pool(name="opsum", bufs=2, space="PSUM") as opsp,
        ):
            xc = constp.tile([128, 2 * NSLOT], f16)
            cstA = constp.tile([GB, CCA], f16)
            cstW = constp.tile([128, NG], f16)
            b2r = constp.tile([1, NG], f32)

            # sync ring: all of x in one big-line descriptor, then the w1
            # block slabs (uniform ~5KB lines; the stream is consumed in
            # exactly this order)
            nc.sync.dma_start(out=xc, in_=xT_d)
            w1tiles = {}
            w1off = 0
            for b, gbt in enumerate(sizes):
                ln = 128 * gbt * 2 * 128
                w1t = w1p.tile([128, gbt, 2, 128], wdt, tag=f"w1t{b}")
                blk = w1_d[w1off:w1off + ln].rearrange(
                    "(p g c h) -> p g c h", p=128, g=gbt, c=2
                )
                q0 = 0
                while q0 < gbt:
                    q1 = min(q0 + MAXG, gbt)
                    nc.sync.dma_start(out=w1t[:, q0:q1], in_=blk[:, q0:q1])
                    q0 = q1
                w1tiles[b] = w1t
                w1off += ln

            # scalar ring, in parallel: just the small constants (cstW
            # feeds the first muls, cstA the first bias-matmul at block 2)
            nc.scalar.dma_start(out=cstW, in_=cstW_d)
            nc.scalar.dma_start(out=b2r, in_=b2_d)
            nc.scalar.dma_start(out=cstA, in_=cstA_d)

            cones = constp.tile([128, 1], f16)
            nc.vector.memset(cones, 1.0)

            out_sb = constp.tile([1, NSLOT], f32)
            prodt = constp.tile([128, NSLOT], f16)

            # output chunks: <=512 cols each, the final boundary snapped to
            # the last (tiny) block so the tail chunk is 40 cols
            last_base = NSLOT - sizes[-1] * W
            nch = -(-last_base // 512)
            chunk = -(-last_base // nch // W) * W
            bounds = []
            lo = 0
            while lo < last_base:
                bounds.append((lo, min(lo + chunk, last_base)))
                lo += chunk
            bounds.append((last_base, NSLOT))

            def _reduce_chunk(lo, hi, early):
                n = hi - lo
                ops = opsp.tile([1, n], f32, tag="ops")
                nc.tensor.matmul(
                    ops, lhsT=cones, rhs=prodt[:, lo:hi], start=True, stop=True
                )
                glo, ghi = lo // W, hi // W
                b2v = b2r[:, glo:ghi].unsqueeze(2).broadcast_to([1, ghi - glo, W])
                nc.vector.tensor_add(
                    out_sb[:, lo:hi].rearrange("p (g w) -> p g w", w=W),
                    ops.rearrange("p (g w) -> p g w", w=W),
                    b2v,
                )
                if early:
                    nc.gpsimd.dma_start(out=out_d[:, lo:hi], in_=out_sb[:, lo:hi])

            for b, gbt in enumerate(sizes):
                g0 = sum(sizes[:b])
                cols = gbt * W
                base = g0 * W
                w1t = w1tiles[b]
                xc0 = xc[:, 2 * base:2 * base + cols]
                xc1 = xc[:, 2 * base + cols:2 * base + 2 * cols]

                ps = hpsp.tile([128, cols], f32, tag="hps")
                # b1 lands in PSUM first (start=True clears the whole zero
                # region, so the block-wide write must precede the per-group
                # accumulations): b1blk.T @ one-hot
                nc.tensor.matmul(
                    ps,
                    lhsT=cstA[0:gbt, b1off(b):b1off(b) + HIDDEN],
                    rhs=cstA[0:gbt, 0:cols],
                    start=True, stop=False, skip_group_check=True,
                )
                for jj in range(gbt):
                    sl = slice(jj * W, (jj + 1) * W)
                    last = jj == gbt - 1
                    nc.tensor.matmul(
                        ps[:, sl], lhsT=w1t[:, jj, 0], rhs=xc0[:, sl],
                        start=False, stop=False, skip_group_check=True,
                    )
                    nc.tensor.matmul(
                        ps[:, sl], lhsT=w1t[:, jj, 1], rhs=xc1[:, sl],
                        start=False, stop=last, skip_group_check=True,
                    )

                # output chunks whose prodt region is already final (all
                # covering blocks' epilogues done at least a block ago) are
                # reduced here, overlapped with the stream, writeback on the
                # idle gpsimd SWDGE — only the final chunk stays in the tail
                while bounds and bounds[0][1] <= base:
                    clo, chi = bounds.pop(0)
                    _reduce_chunk(clo, chi, True)

                # epilogue: gelu straight off PSUM (fp16 out), *w2 on DVE
                # (16-bit, 2x rate) into the resident prodt slab.  The two
                # head blocks run before cstA lands, so their b1 is added
                # by a small STT (b1T rides in cstW's tail columns).
                esb = workp.tile([128, cols], f16, tag="esb")
                halves = [(0, gbt // 2), (gbt // 2, gbt)] if gbt > 15 else [(0, gbt)]
                for ga, gz in halves:
                    hsl = slice(ga * W, gz * W)
                    n_g = gz - ga
                    nc.scalar.activation(esb[:, hsl], ps[:, hsl], gelu)
                    w2v = (
                        cstW[:, g0 + ga:g0 + gz]
                        .unsqueeze(2).broadcast_to([128, n_g, W])
                    )
                    nc.vector.tensor_mul(
                        prodt[:, base + ga * W:base + gz * W]
                        .rearrange("p (g w) -> p g w", w=W),
                        esb[:, hsl].rearrange("p (g w) -> p g w", w=W),
                        w2v,
                    )

            # tail: the remaining chunk(s) + one small sync writeback
            tlo = bounds[0][0]
            for clo, chi in bounds:
                _reduce_chunk(clo, chi, False)
            nc.sync.dma_start(out=out_d[:, tlo:], in_=out_sb[:, tlo:])

    nc.compile()
    return nc


def _get_program(NG, w1_dtype):
    key = (NG, w1_dtype)
    if key not in _PROGRAM_CACHE:
        _PROGRAM_CACHE[key] = _build_program(NG, w1_dtype)
    return _PROGRAM_CACHE[key]


def kernel(x, task_ids, l1_emb, l1_bias, l2_emb, l2_bias):
    global LAST_RESULTS
    x = np.ascontiguousarray(np.asarray(x, dtype=np.float32))
    tid = np.asarray(task_ids).astype(np.int64)
    l1_emb = np.ascontiguousarray(np.asarray(l1_emb, dtype=np.float32))
    l1_bias = np.ascontiguousarray(np.asarray(l1_bias, dtype=np.float32))
    l2_emb = np.ascontiguousarray(np.asarray(l2_emb, dtype=np.float32))
    l2_bias = np.ascontiguousarray(np.asarray(l2_bias, dtype=np.float32))

    B = x.shape[0]
    assert x.shape == (BATCH, N_FEATURES) and tid.shape == (BATCH,)

    wdt = _np_dtype(W1_DTYPE)

    # A "group" is (task, slice of up to W of its samples).  Tasks with more
    # than W samples get several groups; tasks with no samples get none.
    counts = np.bincount(tid, minlength=NUM_TASKS)
    ngroups = (-(-counts // W)).astype(np.int64)  # ceil, 0 for empty tasks
    ng_core = ngroups.reshape(N_CORES, TPC).sum(axis=1)
    NG = -(-int(ng_core.max()) // 5) * 5  # round up to a multiple of 5
    NSLOT = NG * W

    # within-core group base of each task
    gbase = np.empty(NUM_TASKS, dtype=np.int64)
    for c in range(N_CORES):
        sl = slice(c * TPC, (c + 1) * TPC)
        cs = np.cumsum(ngroups[sl])
        gbase[sl] = cs - ngroups[sl]

    # slot routing: sample j -> (core, slot)
    order = np.argsort(tid, kind="stable")
    sorted_tid = tid[order]
    starts = np.flatnonzero(np.r_[True, np.diff(sorted_tid) != 0])
    run_len = np.diff(np.r_[starts, B])
    run_pos = np.arange(B) - np.repeat(starts, run_len)
    occ = np.empty(B, dtype=np.int64)
    occ[order] = run_pos
    core = tid // TPC
    slot = (gbase[tid] + occ // W) * W + occ % W

    # scatter x into per-core transposed, padded slot grids
    xT = np.zeros((N_CORES, N_FEATURES, NSLOT), dtype=np.float16)
    xT[core, :, slot] = x.astype(np.float16)

    sizes = _block_sizes(NG)
    NB = len(sizes)
    IND = GB * W
    CCA = IND + NB * HIDDEN

    # indicator: ind[g, col] = 1.0 where col // W == g
    ind = np.zeros((GB, IND), dtype=np.float16)
    ind[np.arange(IND) // W, np.arange(IND)] = 1.0

    in_maps = []
    for c in range(N_CORES):
        t0 = c * TPC
        sl = slice(t0, t0 + TPC)
        # task id of each group (padded to NG with the core's first task)
        gtask = np.repeat(np.arange(t0, t0 + TPC), ngroups[sl])
        if len(gtask) < NG:
            gtask = np.r_[gtask, np.full(NG - len(gtask), t0)]
        rows = l1_emb[gtask]  # [NG, 32768]
        cstA = np.zeros((GB, CCA), dtype=np.float16)
        cstA[:, :IND] = ind
        # pack w1 per block: [gbt, 2, 128, 128] -> [128, gbt, 2, 128] flat
        parts = []
        cum = 0
        for b, gbt in enumerate(sizes):
            blk = rows[cum:cum + gbt]
            blk = blk.reshape(gbt, 2, 128, 128).transpose(2, 0, 1, 3)
            parts.append(blk.astype(wdt).reshape(-1))
            cstA[0:gbt, IND + b * HIDDEN:IND + (b + 1) * HIDDEN] = (
                l1_bias[gtask[cum:cum + gbt]]
            )
            cum += gbt
        # block-major x: per block [half0 cols | half1 cols] contiguously
        xTc = xT[c].reshape(2, 128, NSLOT)
        xparts = []
        cum2 = 0
        for gbt in sizes:
            cb = gbt * W
            xparts.append(xTc[0][:, cum2:cum2 + cb])
            xparts.append(xTc[1][:, cum2:cum2 + cb])
            cum2 += cb
        in_maps.append({
            "xTb": np.ascontiguousarray(np.concatenate(xparts, axis=1)),
            "w1s": np.concatenate(parts),
            "cstA": cstA,
            "cstW": np.ascontiguousarray(l2_emb[gtask].T.astype(np.float16)),
            "b2r": np.ascontiguousarray(l2_bias[gtask].reshape(1, NG)),
        })

    nc = _get_program(NG, W1_DTYPE)
    if SIM_CORES is not None:
        from concourse.bass_interp import CoreSim

        sim_results = []
        for c in range(N_CORES):
            if c in SIM_CORES:
                kw = {}
                if SIM_EXECUTOR_CLS is not None:
                    kw["executor_cls"] = SIM_EXECUTOR_CLS
                sim = CoreSim(nc, publish_trace=False, **kw)
                for k, v in in_maps[c].items():
                    sim.tensor(k)[:] = v
                sim.simulate()
                sim_results.append({"out": np.array(sim.tensor("out"))})
            else:
                sim_results.append({"out": np.zeros((1, NSLOT), np.float32)})
        outs = np.stack([r["out"].reshape(NSLOT) for r in sim_results])
        logits = outs[core, slot]
        return logits[:, None].astype(np.float32)

    res = run_bass_kernel_spmd(
        nc, in_maps, core_ids=list(range(N_CORES)), trace=TRACE, tmpdir=TMPDIR,
    )
    LAST_RESULTS = res

    outs = np.stack([r["out"].reshape(NSLOT) for r in res.results])
    logits = outs[core, slot]
    return logits[:, None].astype(np.float32)


# revision 15
# speedup vs baseline: 1.0636x; 1.0636x over previous
"""Trainium2 Bass kernel for the per-task embedding MLP (embedding_lookup).

Computation (per sample j with task t = task_ids[j]):
    h      = x[j] @ l1_emb[t].reshape(256, 128) + l1_bias[t]
    g      = gelu_exact(h)
    out[j] = sum(g * l2_emb[t]) + l2_bias[t, 0]

Strategy: shard the *task* axis across the 8 cores (125 contiguous tasks per
core), so each core streams its slab of l1_emb exactly once (the memory
roofline).  Samples are routed (host-side index math only) to the core
owning their task and packed into a slot grid of W=8 columns per group
(tasks with more than W samples get extra groups with duplicated weight
rows; tasks with no samples get none), so all 8 cores run one identical
SPMD program: per group, two K=128 matmuls of the task's [256,128] fp8
weights against its [256,W] fp16 x-columns accumulate hT[128, cols] in
PSUM; a per-block "bias matmul" (block's l1_bias rows against a one-hot
group->slot indicator) accumulates b1 into the same PSUM, so the epilogue
is just gelu (ACT from PSUM, fp16 out) -> *w2 (DVE, 16-bit) into a
resident prodt slab; the hidden-dim reduce (ones-vector matmul), +b2 and
the single output DMA all happen once at the end (keeping them out of the
per-block stream avoids PE-FIFO bubbles and semaphore churn).

Precision: w1 is host-cast to fp8 E3M4 (float8e3) — halves the dominant
l1_emb DMA stream vs fp16; x stays fp16 (mixed-dtype matmuls are fine).
b1/w2 ride in fp16, b2 in fp32, PSUM accumulation in fp32.  Measured
end-to-end L2 relative error ~1.3e-2 (dominated by the 4-bit-mantissa
weight quantization), within the 2e-2 gate.

DMA plan: a dma_start costs ~0.6us of issuing-engine time and small-line
descriptors are starved by the per-engine packet round-robin, so
descriptors are few and large.  x is packed block-major (both K-halves of
a block contiguous -> one descriptor per block, 4*cols-byte lines).  The
sync HWDGE ring carries the tiny x lead (blocks 0-1) then the w1 slabs;
the scalar ring carries the packed constants and the x slabs for blocks
2+ in parallel.  All block tiles are SBUF/PSUM-resident (bufs=NB), so no
descriptor ever waits on compute progress.
"""

import numpy as np
import ml_dtypes

import concourse.bacc as bacc
import concourse.mybir as mybir
import concourse.tile as tile
from concourse.bass_utils import run_bass_kernel_spmd

NUM_TASKS = 1000
N_FEATURES = 256
HIDDEN = 128
BATCH = 4096
N_CORES = 8
TPC = NUM_TASKS // N_CORES  # tasks per core = 125

# Module-level knobs for the test harness (the grader just calls kernel()).
W1_DTYPE = "float8e3"  # w1 slab dtype: "float8e3" | "float16"
TRACE = False
TMPDIR = None  # optional fixed artifact dir for profiling runs
SIM_CORES = None  # e.g. [0]: run CoreSim for those cores instead of hardware
SIM_EXECUTOR_CLS = None  # optional InstructionExecutor subclass for CoreSim
LAST_RESULTS = None

_PROGRAM_CACHE = {}

W = 8           # sample slots per group
GB = 60         # max groups per PSUM block (GB*W*4B <= 2KB bank)
MAXG = 20       # max groups per w1 DMA descriptor (5KB lines)


def _np_dtype(name):
    return {
        "float8e3": ml_dtypes.float8_e3m4,
        "float16": np.float16,
        "float32": np.float32,
    }[name]


def _block_sizes(NG):
    # the kernel is DMA-bound: big blocks first (fewer descriptors, fewer
    # ops), a short ramp-down at the end so the post-last-matmul epilogue
    # chain is cheap
    rem = NG - 20
    sizes = [GB] * (rem // GB) + ([rem % GB] if rem % GB else []) + [15, 5]
    assert sum(sizes) == NG and all(s <= GB for s in sizes)
    return sizes


def _build_program(NG, w1_dtype):
    sizes = _block_sizes(NG)
    NB = len(sizes)
    NSLOT = NG * W
    f32 = mybir.dt.float32
    f16 = mybir.dt.float16
    wdt = getattr(mybir.dt, w1_dtype)

    IND = GB * W                      # indicator columns in cstA
    CCA = IND + NB * HIDDEN           # cstA fp16 columns

    def b1off(b):
        return IND + b * HIDDEN

    nc = bacc.Bacc("TRN2", target_bir_lowering=False, debug=False)

    # x is packed block-major: for block b both K-halves sit contiguously
    # ([.., half0 cols, half1 cols, ..]), so one descriptor per block with
    # 4*cols-byte lines
    xT_d = nc.dram_tensor("xTb", [128, 2 * NSLOT], f16, kind="ExternalInput").ap()
    w1_d = nc.dram_tensor(
        "w1s", [NG * N_FEATURES * HIDDEN], wdt, kind="ExternalInput"
    ).ap()
    cstA_d = nc.dram_tensor("cstA", [GB, CCA], f16, kind="ExternalInput").ap()
    cstW_d = nc.dram_tensor("cstW", [128, NG], f16, kind="ExternalInput").ap()
    b2_d = nc.dram_tensor("b2r", [1, NG], f32, kind="ExternalInput").ap()
    out_d = nc.dram_tensor("out", [1, NSLOT], f32, kind="ExternalOutput").ap()

    gelu = mybir.ActivationFunctionType.Gelu

    with tile.TileContext(nc) as tc:
        with (
            tc.tile_pool(name="const", bufs=1) as constp,
            tc.tile_pool(name="w1pool", bufs=1) as w1p,
            tc.tile_pool(name="work", bufs=3) as workp,
            tc.tile_pool(name="hpsum", bufs=NB, space="PSUM") as hpsp,
            tc.tile_pool(name="opsum", bufs=2, space="PSUM") as opsp,
        ):
            xc = constp.tile([128, 2 * NSLOT], f16)
            cstA = constp.tile([GB, CCA], f16)
            cstW = constp.tile([128, NG], f16)
            b2r = constp.tile([1, NG], f32)

            # sync ring: all of x in one big-line descriptor, then the w1
            # block slabs (uniform ~5KB lines; the stream is consumed in
            # exactly this order)
            nc.sync.dma_start(out=xc, in_=xT_d)
            w1tiles = {}
            w1off = 0
            for b, gbt in enumerate(sizes):
                ln = 128 * gbt * 2 * 128
                w1t = w1p.tile([128, gbt, 2, 128], wdt, tag=f"w1t{b}")
                blk = w1_d[w1off:w1off + ln].rearrange(
                    "(p g c h) -> p g c h", p=128, g=gbt, c=2
                )
                q0 = 0
                while q0 < gbt:
                    q1 = min(q0 + MAXG, gbt)
                    nc.sync.dma_start(out=w1t[:, q0:q1], in_=blk[:, q0:q1])
                    q0 = q1
                w1tiles[b] = w1t
                w1off += ln

            # scalar ring, in parallel: just the small constants (cstW
            # feeds the first muls, cstA the first bias-matmul at block 2)
            nc.scalar.dma_start(out=cstW, in_=cstW_d)
            nc.scalar.dma_start(out=b2r, in_=b2_d)
            nc.scalar.dma_start(out=cstA, in_=cstA_d)

            cones = constp.tile([128, 1], f16)
            nc.vector.memset(cones, 1.0)

            out_sb = constp.tile([1, NSLOT], f32)
            prodt = constp.tile([128, NSLOT], f16)

            # output chunks: <=512 cols each, the final boundary snapped to
            # the last (tiny) block so the tail chunk is 40 cols
            last_base = NSLOT - sizes[-1] * W
            nch = -(-last_base // 512)
            chunk = -(-last_base // nch // W) * W
            bounds = []
            lo = 0
            while lo < last_base:
                bounds.append((lo, min(lo + chunk, last_base)))
                lo += chunk
            bounds.append((last_base, NSLOT))

            def _reduce_chunk(lo, hi, early):
                n = hi - lo
                ops = opsp.tile([1, n], f32, tag="ops")
                nc.tensor.matmul(
                    ops, lhsT=cones, rhs=prodt[:, lo:hi], start=True, stop=True
                )
                glo, ghi = lo // W, hi // W
                b2v = b2r[:, glo:ghi].unsqueeze(2).broadcast_to([1, ghi - glo, W])
                nc.vector.tensor_add(
                    out_sb[:, lo:hi].rearrange("p (g w) -> p g w", w=W),
                    ops.rearrange("p (g w) -> p g w", w=W),
                    b2v,
                )
                if early:
                    nc.gpsimd.dma_start(out=out_d[:, lo:hi], in_=out_sb[:, lo:hi])

            from contextlib import ExitStack as _ES

            for b, gbt in enumerate(sizes):
                # sim-time gate: forces the static per-engine schedule to
                # follow block order (the compile-time scheduler otherwise
                # interleaves blocks' matmuls, pushing each epilogue's
                # semaphore threshold deep into later blocks).  DMA
                # descriptors are emitted before the loop with no deps on
                # compute, so the gates cannot stall the stream.
                _g = _ES(); _g.enter_context(tc.tile_wait_until(b + 1))
                g0 = sum(sizes[:b])
                cols = gbt * W
                base = g0 * W
                w1t = w1tiles[b]
                xc0 = xc[:, 2 * base:2 * base + cols]
                xc1 = xc[:, 2 * base + cols:2 * base + 2 * cols]

                ps = hpsp.tile([128, cols], f32, tag="hps")
                # b1 lands in PSUM first (start=True clears the whole zero
                # region, so the block-wide write must precede the per-group
                # accumulations): b1blk.T @ one-hot
                nc.tensor.matmul(
                    ps,
                    lhsT=cstA[0:gbt, b1off(b):b1off(b) + HIDDEN],
                    rhs=cstA[0:gbt, 0:cols],
                    start=True, stop=False, skip_group_check=True,
                )
                for jj in range(gbt):
                    sl = slice(jj * W, (jj + 1) * W)
                    last = jj == gbt - 1
                    nc.tensor.matmul(
                        ps[:, sl], lhsT=w1t[:, jj, 0], rhs=xc0[:, sl],
                        start=False, stop=False, skip_group_check=True,
                    )
                    nc.tensor.matmul(
                        ps[:, sl], lhsT=w1t[:, jj, 1], rhs=xc1[:, sl],
                        start=False, stop=last, skip_group_check=True,
                    )

                # output chunks whose prodt region is already final (all
                # covering blocks' epilogues done at least a block ago) are
                # reduced here, overlapped with the stream, writeback on the
                # idle gpsimd SWDGE — only the final chunk stays in the tail
                while bounds and bounds[0][1] <= base:
                    clo, chi = bounds.pop(0)
                    _reduce_chunk(clo, chi, True)
                _gate_close = _g.close

                # epilogue: gelu straight off PSUM (fp16 out), *w2 on DVE
                # (16-bit, 2x rate) into the resident prodt slab.  The two
                # head blocks run before cstA lands, so their b1 is added
                # by a small STT (b1T rides in cstW's tail columns).
                esb = workp.tile([128, cols], f16, tag="esb")
                halves = [(0, gbt // 2), (gbt // 2, gbt)] if gbt > 15 else [(0, gbt)]
                for ga, gz in halves:
                    hsl = slice(ga * W, gz * W)
                    n_g = gz - ga
                    nc.scalar.activation(esb[:, hsl], ps[:, hsl], gelu)
                    w2v = (
                        cstW[:, g0 + ga:g0 + gz]
                        .unsqueeze(2).broadcast_to([128, n_g, W])
                    )
                    nc.vector.tensor_mul(
                        prodt[:, base + ga * W:base + gz * W]
                        .rearrange("p (g w) -> p g w", w=W),
                        esb[:, hsl].rearrange("p (g w) -> p g w", w=W),
                        w2v,
                    )
                _gate_close()

            # tail: the remaining chunk(s) + one small sync writeback
            with tc.tile_wait_until(NB + 2):
                tlo = bounds[0][0]
                for clo, chi in bounds:
                    _reduce_chunk(clo, chi, False)
                nc.sync.dma_start(out=out_d[:, tlo:], in_=out_sb[:, tlo:])

    nc.compile()
    return nc


def _get_program(NG, w1_dtype):
    key = (NG, w1_dtype)
    if key not in _PROGRAM_CACHE:
        _PROGRAM_CACHE[key] = _build_program(NG, w1_dtype)
    return _PROGRAM_CACHE[key]


def kernel(x, task_ids, l1_emb, l1_bias, l2_emb, l2_bias):
    global LAST_RESULTS
    x = np.ascontiguousarray(np.asarray(x, dtype=np.float32))
    tid = np.asarray(task_ids).astype(np.int64)
    l1_emb = np.ascontiguousarray(np.asarray(l1_emb, dtype=np.float32))
    l1_bias = np.ascontiguousarray(np.asarray(l1_bias, dtype=np.float32))
    l2_emb = np.ascontiguousarray(np.asarray(l2_emb, dtype=np.float32))
    l2_bias = np.ascontiguousarray(np.asarray(l2_bias, dtype=np.float32))

    B = x.shape[0]
    assert x.shape == (BATCH, N_FEATURES) and tid.shape == (BATCH,)

    wdt = _np_dtype(W1_DTYPE)

    # A "group" is (task, slice of up to W of its samples).  Tasks with more
    # than W samples get several groups; tasks with no samples get none.
    counts = np.bincount(tid, minlength=NUM_TASKS)
    ngroups = (-(-counts // W)).astype(np.int64)  # ceil, 0 for empty tasks
    ng_core = ngroups.reshape(N_CORES, TPC).sum(axis=1)
    NG = -(-int(ng_core.max()) // 5) * 5  # round up to a multiple of 5
    NSLOT = NG * W

    # within-core group base of each task
    gbase = np.empty(NUM_TASKS, dtype=np.int64)
    for c in range(N_CORES):
        sl = slice(c * TPC, (c + 1) * TPC)
        cs = np.cumsum(ngroups[sl])
        gbase[sl] = cs - ngroups[sl]

    # slot routing: sample j -> (core, slot)
    order = np.argsort(tid, kind="stable")
    sorted_tid = tid[order]
    starts = np.flatnonzero(np.r_[True, np.diff(sorted_tid) != 0])
    run_len = np.diff(np.r_[starts, B])
    run_pos = np.arange(B) - np.repeat(starts, run_len)
    occ = np.empty(B, dtype=np.int64)
    occ[order] = run_pos
    core = tid // TPC
    slot = (gbase[tid] + occ // W) * W + occ % W

    # scatter x into per-core transposed, padded slot grids
    xT = np.zeros((N_CORES, N_FEATURES, NSLOT), dtype=np.float16)
    xT[core, :, slot] = x.astype(np.float16)

    sizes = _block_sizes(NG)
    NB = len(sizes)
    IND = GB * W
    CCA = IND + NB * HIDDEN

    # indicator: ind[g, col] = 1.0 where col // W == g
    ind = np.zeros((GB, IND), dtype=np.float16)
    ind[np.arange(IND) // W, np.arange(IND)] = 1.0

    in_maps = []
    for c in range(N_CORES):
        t0 = c * TPC
        sl = slice(t0, t0 + TPC)
        # task id of each group (padded to NG with the core's first task)
        gtask = np.repeat(np.arange(t0, t0 + TPC), ngroups[sl])
        if len(gtask) < NG:
            gtask = np.r_[gtask, np.full(NG - len(gtask), t0)]
        rows = l1_emb[gtask]  # [NG, 32768]
        cstA = np.zeros((GB, CCA), dtype=np.float16)
        cstA[:, :IND] = ind
        # pack w1 per block: [gbt, 2, 128, 128] -> [128, gbt, 2, 128] flat
        parts = []
        cum = 0
        for b, gbt in enumerate(sizes):
            blk = rows[cum:cum + gbt]
            blk = blk.reshape(gbt, 2, 128, 128).transpose(2, 0, 1, 3)
            parts.append(blk.astype(wdt).reshape(-1))
            cstA[0:gbt, IND + b * HIDDEN:IND + (b + 1) * HIDDEN] = (
                l1_bias[gtask[cum:cum + gbt]]
            )
            cum += gbt
        # block-major x: per block [half0 cols | half1 cols] contiguously
        xTc = xT[c].reshape(2, 128, NSLOT)
        xparts = []
        cum2 = 0
        for gbt in sizes:
            cb = gbt * W
            xparts.append(xTc[0][:, cum2:cum2 + cb])
            xparts.append(xTc[1][:, cum2:cum2 + cb])
            cum2 += cb
        in_maps.append({
            "xTb": np.ascontiguousarray(np.concatenate(xparts, axis=1)),
            "w1s": np.concatenate(parts),
            "cstA": cstA,
            "cstW": np.ascontiguousarray(l2_emb[gtask].T.astype(np.float16)),
            "b2r": np.ascontiguousarray(l2_bias[gtask].reshape(1, NG)),
        })

    nc = _get_program(NG, W1_DTYPE)
    if SIM_CORES is not None:
        from concourse.bass_interp import CoreSim

        sim_results = []
        for c in range(N_CORES):
            if c in SIM_CORES:
                kw = {}
                if SIM_EXECUTOR_CLS is not None:
                    kw["executor_cls"] = SIM_EXECUTOR_CLS
                sim = CoreSim(nc, publish_trace=False, **kw)
                for k, v in in_maps[c].items():
                    sim.tensor(k)[:] = v
                sim.simulate()
                sim_results.append({"out": np.array(sim.tensor("out"))})
            else:
                sim_results.append({"out": np.zeros((1, NSLOT), np.float32)})
        outs = np.stack([r["out"].reshape(NSLOT) for r in sim_results])
        logits = outs[core, slot]
        return logits[:, None].astype(np.float32)

    res = run_bass_kernel_spmd(
        nc, in_maps, core_ids=list(range(N_CORES)), trace=TRACE, tmpdir=TMPDIR,
    )
    LAST_RESULTS = res

    outs = np.stack([r["out"].reshape(NSLOT) for r in res.results])
    logits = outs[core, slot]
    return logits[:, None].astype(np.float32)


# revision 16
# speedup vs baseline: 1.0654x; 1.0017x over previous
"""Trainium2 Bass kernel for the per-task embedding MLP (embedding_lookup).

Computation (per sample j with task t = task_ids[j]):
    h      = x[j] @ l1_emb[t].reshape(256, 128) + l1_bias[t]
    g      = gelu_exact(h)
    out[j] = sum(g * l2_emb[t]) + l2_bias[t, 0]

Strategy: shard the *task* axis across the 8 cores (125 contiguous tasks per
core), so each core streams its slab of l1_emb exactly once (the memory
roofline).  Samples are routed (host-side index math only) to the core
owning their task and packed into a slot grid of W=8 columns per group
(tasks with more than W samples get extra groups with duplicated weight
rows; tasks with no samples get none), so all 8 cores run one identical
SPMD program: per group, two K=128 matmuls of the task's [256,128] fp8
weights against its [256,W] fp16 x-columns accumulate hT[128, cols] in
PSUM; a per-block "bias matmul" (block's l1_bias rows against a one-hot
group->slot indicator) accumulates b1 into the same PSUM, so the epilogue
is just gelu (ACT from PSUM, fp16 out) -> *w2 (DVE, 16-bit) into a
resident prodt slab; the hidden-dim reduce (ones-vector matmul), +b2 and
the single output DMA all happen once at the end (keeping them out of the
per-block stream avoids PE-FIFO bubbles and semaphore churn).

Precision: w1 is host-cast to fp8 E3M4 (float8e3) — halves the dominant
l1_emb DMA stream vs fp16; x stays fp16 (mixed-dtype matmuls are fine).
b1/w2 ride in fp16, b2 in fp32, PSUM accumulation in fp32.  Measured
end-to-end L2 relative error ~1.3e-2 (dominated by the 4-bit-mantissa
weight quantization), within the 2e-2 gate.

DMA plan: a dma_start costs ~0.6us of issuing-engine time and small-line
descriptors are starved by the per-engine packet round-robin, so
descriptors are few and large.  x is packed block-major (both K-halves of
a block contiguous -> one descriptor per block, 4*cols-byte lines).  The
sync HWDGE ring carries the tiny x lead (blocks 0-1) then the w1 slabs;
the scalar ring carries the packed constants and the x slabs for blocks
2+ in parallel.  All block tiles are SBUF/PSUM-resident (bufs=NB), so no
descriptor ever waits on compute progress.
"""

import numpy as np
import ml_dtypes

import concourse.bacc as bacc
import concourse.mybir as mybir
import concourse.tile as tile
from concourse.bass_utils import run_bass_kernel_spmd

NUM_TASKS = 1000
N_FEATURES = 256
HIDDEN = 128
BATCH = 4096
N_CORES = 8
TPC = NUM_TASKS // N_CORES  # tasks per core = 125

# Module-level knobs for the test harness (the grader just calls kernel()).
W1_DTYPE = "float8e3"  # w1 slab dtype: "float8e3" | "float16"
TRACE = False
TMPDIR = None  # optional fixed artifact dir for profiling runs
SIM_CORES = None  # e.g. [0]: run CoreSim for those cores instead of hardware
SIM_EXECUTOR_CLS = None  # optional InstructionExecutor subclass for CoreSim
LAST_RESULTS = None

_PROGRAM_CACHE = {}

W = 8           # sample slots per group
GB = 60         # max groups per PSUM block (GB*W*4B <= 2KB bank)
MAXG = 20       # max groups per w1 DMA descriptor (5KB lines)


def _np_dtype(name):
    return {
        "float8e3": ml_dtypes.float8_e3m4,
        "float16": np.float16,
        "float32": np.float32,
    }[name]


def _block_sizes(NG):
    # the kernel is DMA-bound: big blocks first (fewer descriptors, fewer
    # ops), a short ramp-down at the end so the post-last-matmul epilogue
    # chain is cheap
    rem = NG - 20
    sizes = [GB] * (rem // GB) + ([rem % GB] if rem % GB else []) + [20]
    assert sum(sizes) == NG and all(s <= GB for s in sizes)
    return sizes


def _build_program(NG, w1_dtype):
    sizes = _block_sizes(NG)
    NB = len(sizes)
    NSLOT = NG * W
    f32 = mybir.dt.float32
    f16 = mybir.dt.float16
    wdt = getattr(mybir.dt, w1_dtype)

    IND = GB * W                      # indicator columns in cstA
    CCA = IND + NB * HIDDEN           # cstA fp16 columns

    def b1off(b):
        return IND + b * HIDDEN

    nc = bacc.Bacc("TRN2", target_bir_lowering=False, debug=False)

    # x is packed block-major: for block b both K-halves sit contiguously
    # ([.., half0 cols, half1 cols, ..]), so one descriptor per block with
    # 4*cols-byte lines
    xT_d = nc.dram_tensor("xTb", [128, 2 * NSLOT], f16, kind="ExternalInput").ap()
    w1_d = nc.dram_tensor(
        "w1s", [NG * N_FEATURES * HIDDEN], wdt, kind="ExternalInput"
    ).ap()
    cstA_d = nc.dram_tensor("cstA", [GB, CCA], f16, kind="ExternalInput").ap()
    cstW_d = nc.dram_tensor("cstW", [128, NG], f16, kind="ExternalInput").ap()
    b2_d = nc.dram_tensor("b2r", [1, NG], f32, kind="ExternalInput").ap()
    out_d = nc.dram_tensor("out", [1, NSLOT], f32, kind="ExternalOutput").ap()

    gelu = mybir.ActivationFunctionType.Gelu

    with tile.TileContext(nc) as tc:
        with (
            tc.tile_pool(name="const", bufs=1) as constp,
            tc.tile_pool(name="w1pool", bufs=1) as w1p,
            tc.tile_pool(name="work", bufs=3) as workp,
            tc.tile_pool(name="hpsum", bufs=NB, space="PSUM") as hpsp,
            tc.tile_pool(name="opsum", bufs=2, space="PSUM") as opsp,
        ):
            xc = constp.tile([128, 2 * NSLOT], f16)
            cstA = constp.tile([GB, CCA], f16)
            cstW = constp.tile([128, NG], f16)
            b2r = constp.tile([1, NG], f32)

            # sync ring: all of x in one big-line descriptor, then the w1
            # block slabs (uniform ~5KB lines; the stream is consumed in
            # exactly this order)
            nc.sync.dma_start(out=xc, in_=xT_d)
            w1tiles = {}
            w1off = 0
            for b, gbt in enumerate(sizes):
                ln = 128 * gbt * 2 * 128
                w1t = w1p.tile([128, gbt, 2, 128], wdt, tag=f"w1t{b}")
                blk = w1_d[w1off:w1off + ln].rearrange(
                    "(p g c h) -> p g c h", p=128, g=gbt, c=2
                )
                q0 = 0
                while q0 < gbt:
                    q1 = min(q0 + MAXG, gbt)
                    nc.sync.dma_start(out=w1t[:, q0:q1], in_=blk[:, q0:q1])
                    q0 = q1
                w1tiles[b] = w1t
                w1off += ln

            # scalar ring, in parallel: just the small constants (cstW
            # feeds the first muls, cstA the first bias-matmul at block 2)
            nc.scalar.dma_start(out=cstW, in_=cstW_d)
            nc.scalar.dma_start(out=b2r, in_=b2_d)
            nc.scalar.dma_start(out=cstA, in_=cstA_d)

            cones = constp.tile([128, 1], f16)
            nc.vector.memset(cones, 1.0)

            out_sb = constp.tile([1, NSLOT], f32)
            prodt = constp.tile([128, NSLOT], f16)

            # output chunks: <=512 cols each, the final boundary snapped to
            # the last (tiny) block so the tail chunk is 40 cols
            last_base = NSLOT - sizes[-1] * W
            nch = -(-last_base // 512)
            chunk = -(-last_base // nch // W) * W
            bounds = []
            lo = 0
            while lo < last_base:
                bounds.append((lo, min(lo + chunk, last_base)))
                lo += chunk
            bounds.append((last_base, NSLOT))

            def _reduce_chunk(lo, hi, early):
                n = hi - lo
                ops = opsp.tile([1, n], f32, tag="ops")
                nc.tensor.matmul(
                    ops, lhsT=cones, rhs=prodt[:, lo:hi], start=True, stop=True
                )
                glo, ghi = lo // W, hi // W
                b2v = b2r[:, glo:ghi].unsqueeze(2).broadcast_to([1, ghi - glo, W])
                nc.vector.tensor_add(
                    out_sb[:, lo:hi].rearrange("p (g w) -> p g w", w=W),
                    ops.rearrange("p (g w) -> p g w", w=W),
                    b2v,
                )
                if early:
                    nc.gpsimd.dma_start(out=out_d[:, lo:hi], in_=out_sb[:, lo:hi])

            from contextlib import ExitStack as _ES

            for b, gbt in enumerate(sizes):
                # sim-time gate: forces the static per-engine schedule to
                # follow block order (the compile-time scheduler otherwise
                # interleaves blocks' matmuls, pushing each epilogue's
                # semaphore threshold deep into later blocks).  DMA
                # descriptors are emitted before the loop with no deps on
                # compute, so the gates cannot stall the stream.
                _g = _ES(); _g.enter_context(tc.tile_wait_until(b + 1))
                g0 = sum(sizes[:b])
                cols = gbt * W
                base = g0 * W
                w1t = w1tiles[b]
                xc0 = xc[:, 2 * base:2 * base + cols]
                xc1 = xc[:, 2 * base + cols:2 * base + 2 * cols]

                ps = hpsp.tile([128, cols], f32, tag="hps")
                # b1 lands in PSUM first (start=True clears the whole zero
                # region, so the block-wide write must precede the per-group
                # accumulations): b1blk.T @ one-hot
                nc.tensor.matmul(
                    ps,
                    lhsT=cstA[0:gbt, b1off(b):b1off(b) + HIDDEN],
                    rhs=cstA[0:gbt, 0:cols],
                    start=True, stop=False, skip_group_check=True,
                )
                for jj in range(gbt):
                    sl = slice(jj * W, (jj + 1) * W)
                    last = jj == gbt - 1
                    nc.tensor.matmul(
                        ps[:, sl], lhsT=w1t[:, jj, 0], rhs=xc0[:, sl],
                        start=False, stop=False, skip_group_check=True,
                    )
                    nc.tensor.matmul(
                        ps[:, sl], lhsT=w1t[:, jj, 1], rhs=xc1[:, sl],
                        start=False, stop=last, skip_group_check=True,
                    )

                # output chunks whose prodt region is already final (all
                # covering blocks' epilogues done at least a block ago) are
                # reduced here, overlapped with the stream, writeback on the
                # idle gpsimd SWDGE — only the final chunk stays in the tail
                while bounds and bounds[0][1] <= base:
                    clo, chi = bounds.pop(0)
                    _reduce_chunk(clo, chi, True)
                _gate_close = _g.close

                # epilogue: gelu straight off PSUM (fp16 out), *w2 on DVE
                # (16-bit, 2x rate) into the resident prodt slab.  The two
                # head blocks run before cstA lands, so their b1 is added
                # by a small STT (b1T rides in cstW's tail columns).
                esb = workp.tile([128, cols], f16, tag="esb")
                halves = [(0, gbt // 2), (gbt // 2, gbt)] if gbt > 20 else [(0, gbt)]
                for ga, gz in halves:
                    hsl = slice(ga * W, gz * W)
                    n_g = gz - ga
                    nc.scalar.activation(esb[:, hsl], ps[:, hsl], gelu)
                    w2v = (
                        cstW[:, g0 + ga:g0 + gz]
                        .unsqueeze(2).broadcast_to([128, n_g, W])
                    )
                    nc.vector.tensor_mul(
                        prodt[:, base + ga * W:base + gz * W]
                        .rearrange("p (g w) -> p g w", w=W),
                        esb[:, hsl].rearrange("p (g w) -> p g w", w=W),
                        w2v,
                    )
                _gate_close()

            # tail: the remaining chunk(s) + one small sync writeback
            with tc.tile_wait_until(NB + 2):
                tlo = bounds[0][0]
                for clo, chi in bounds:
                    _reduce_chunk(clo, chi, False)
                nc.sync.dma_start(out=out_d[:, tlo:], in_=out_sb[:, tlo:])

    nc.compile()
    return nc


def _get_program(NG, w1_dtype):
    key = (NG, w1_dtype)
    if key not in _PROGRAM_CACHE:
        _PROGRAM_CACHE[key] = _build_program(NG, w1_dtype)
    return _PROGRAM_CACHE[key]


def kernel(x, task_ids, l1_emb, l1_bias, l2_emb, l2_bias):
    global LAST_RESULTS
    x = np.ascontiguousarray(np.asarray(x, dtype=np.float32))
    tid = np.asarray(task_ids).astype(np.int64)
    l1_emb = np.ascontiguousarray(np.asarray(l1_emb, dtype=np.float32))
    l1_bias = np.ascontiguousarray(np.asarray(l1_bias, dtype=np.float32))
    l2_emb = np.ascontiguousarray(np.asarray(l2_emb, dtype=np.float32))
    l2_bias = np.ascontiguousarray(np.asarray(l2_bias, dtype=np.float32))

    B = x.shape[0]
    assert x.shape == (BATCH, N_FEATURES) and tid.shape == (BATCH,)

    wdt = _np_dtype(W1_DTYPE)

    # A "group" is (task, slice of up to W of its samples).  Tasks with more
    # than W samples get several groups; tasks with no samples get none.
    counts = np.bincount(tid, minlength=NUM_TASKS)
    ngroups = (-(-counts // W)).astype(np.int64)  # ceil, 0 for empty tasks
    ng_core = ngroups.reshape(N_CORES, TPC).sum(axis=1)
    NG = -(-int(ng_core.max()) // 5) * 5  # round up to a multiple of 5
    NSLOT = NG * W

    # within-core group base of each task
    gbase = np.empty(NUM_TASKS, dtype=np.int64)
    for c in range(N_CORES):
        sl = slice(c * TPC, (c + 1) * TPC)
        cs = np.cumsum(ngroups[sl])
        gbase[sl] = cs - ngroups[sl]

    # slot routing: sample j -> (core, slot)
    order = np.argsort(tid, kind="stable")
    sorted_tid = tid[order]
    starts = np.flatnonzero(np.r_[True, np.diff(sorted_tid) != 0])
    run_len = np.diff(np.r_[starts, B])
    run_pos = np.arange(B) - np.repeat(starts, run_len)
    occ = np.empty(B, dtype=np.int64)
    occ[order] = run_pos
    core = tid // TPC
    slot = (gbase[tid] + occ // W) * W + occ % W

    # scatter x into per-core transposed, padded slot grids
    xT = np.zeros((N_CORES, N_FEATURES, NSLOT), dtype=np.float16)
    xT[core, :, slot] = x.astype(np.float16)

    sizes = _block_sizes(NG)
    NB = len(sizes)
    IND = GB * W
    CCA = IND + NB * HIDDEN

    # indicator: ind[g, col] = 1.0 where col // W == g
    ind = np.zeros((GB, IND), dtype=np.float16)
    ind[np.arange(IND) // W, np.arange(IND)] = 1.0

    in_maps = []
    for c in range(N_CORES):
        t0 = c * TPC
        sl = slice(t0, t0 + TPC)
        # task id of each group (padded to NG with the core's first task)
        gtask = np.repeat(np.arange(t0, t0 + TPC), ngroups[sl])
        if len(gtask) < NG:
            gtask = np.r_[gtask, np.full(NG - len(gtask), t0)]
        rows = l1_emb[gtask]  # [NG, 32768]
        cstA = np.zeros((GB, CCA), dtype=np.float16)
        cstA[:, :IND] = ind
        # pack w1 per block: [gbt, 2, 128, 128] -> [128, gbt, 2, 128] flat
        parts = []
        cum = 0
        for b, gbt in enumerate(sizes):
            blk = rows[cum:cum + gbt]
            blk = blk.reshape(gbt, 2, 128, 128).transpose(2, 0, 1, 3)
            parts.append(blk.astype(wdt).reshape(-1))
            cstA[0:gbt, IND + b * HIDDEN:IND + (b + 1) * HIDDEN] = (
                l1_bias[gtask[cum:cum + gbt]]
            )
            cum += gbt
        # block-major x: per block [half0 cols | half1 cols] contiguously
        xTc = xT[c].reshape(2, 128, NSLOT)
        xparts = []
        cum2 = 0
        for gbt in sizes:
            cb = gbt * W
            xparts.append(xTc[0][:, cum2:cum2 + cb])
            xparts.append(xTc[1][:, cum2:cum2 + cb])
            cum2 += cb
        in_maps.append({
            "xTb": np.ascontiguousarray(np.concatenate(xparts, axis=1)),
            "w1s": np.concatenate(parts),
            "cstA": cstA,
            "cstW": np.ascontiguousarray(l2_emb[gtask].T.astype(np.float16)),
            "b2r": np.ascontiguousarray(l2_bias[gtask].reshape(1, NG)),
        })

    nc = _get_program(NG, W1_DTYPE)
    if SIM_CORES is not None:
        from concourse.bass_interp import CoreSim

        sim_results = []
        for c in range(N_CORES):
            if c in SIM_CORES:
                kw = {}
                if SIM_EXECUTOR_CLS is not None:
                    kw["executor_cls"] = SIM_EXECUTOR_CLS
                sim = CoreSim(nc, publish_trace=False, **kw)
                for k, v in in_maps[c].items():
                    sim.tensor(k)[:] = v
                sim.simulate()
                sim_results.append({"out": np.array(sim.tensor("out"))})
            else:
                sim_results.append({"out": np.zeros((1, NSLOT), np.float32)})
        outs = np.stack([r["out"].reshape(NSLOT) for r in sim_results])
        logits = outs[core, slot]
        return logits[:, None].astype(np.float32)

    res = run_bass_kernel_spmd(
        nc, in_maps, core_ids=list(range(N_CORES)), trace=TRACE, tmpdir=TMPDIR,
    )
    LAST_RESULTS = res

    outs = np.stack([r["out"].reshape(NSLOT) for r in res.results])
    logits = outs[core, slot]
    return logits[:, None].astype(np.float32)
